# revision 3
# baseline (speedup 1.0000x reference)
"""GT (graph-transformer) layer on 8 TRN2 NeuronCores via Bass/Tile.

Sharding: node rows split 512/core (the sharding_hint's row sharding).
Each core computes K,V for all 4096 nodes, Q for its own 512, then masked
attention over transposed score tiles:

  - scoresT[m, n_loc] per head via row-tiled PE matmuls (K = head_dim = 32)
  - F = exp(scores) on the ACT engine straight out of PSUM (no masking
    first; masked-softmax rewritten as E = 1 - A + A*F since A is 0/1)
  - G = A * F on DVE; "1 - A" is folded into the AV matmul as
    colsum(V) - V^T A (one extra full-array matmul per head-pair that
    shares the A tile stream)
  - U^T accumulates via 64-wide col-tiled matmuls; a ones-column in V
    yields the softmax denominator Z for free
  - epilogue: divide by Z, Wo, residual, BatchNorm (stats AllGathered
    across the 8 cores), FFN, residual, BatchNorm, output y^T

The host pre-transposes/permutes/bf16-converts all weights so heads come
out block-contiguous (o' = h*32 + d) and the device never transposes.
"""

import sys

sys.path.insert(0, "/opt/trn_rl_repo")

import numpy as np
import ml_dtypes

import concourse.tile as tile
from concourse import mybir, bacc

N = 4096
D = 256
H = 8
HD = 32
NCORES = 8
NLOC = N // NCORES  # 512
MCH = N // 128      # 32 m-chunks
EPS = 1e-5
SCALE = D ** -0.5
P = 128

BF16 = mybir.dt.bfloat16
F32 = mybir.dt.float32
AX = mybir.AxisListType
OP = mybir.AluOpType
ACTF = mybir.ActivationFunctionType


def _bf(x):
    return np.asarray(x, dtype=ml_dtypes.bfloat16)


def _prep_inputs(A, h, Wq, Wk, Wv, Wo, g1, b1, g2, b2, W1, W2):
    A = np.asarray(A, np.float32)
    h = np.asarray(h, np.float32)

    def perm_rows(W):  # head-blocked output channels o' = h*32 + d
        return W.reshape(HD, H, D).transpose(1, 0, 2).reshape(D, D)

    wqT = _bf(perm_rows(np.asarray(Wq, np.float32) * SCALE).T)
    wkT = _bf(perm_rows(np.asarray(Wk, np.float32)).T)
    wvT = _bf(perm_rows(np.asarray(Wv, np.float32)).T)

    Wo_perm = (np.asarray(Wo, np.float32)
               .reshape(D, HD, H).transpose(0, 2, 1).reshape(D, D))
    woT = Wo_perm.T
    woP = np.zeros((4 * 128, D), np.float32)
    for p in range(4):
        woP[p * 128 + 0:p * 128 + 32] = woT[32 * (2 * p):32 * (2 * p) + 32]
        woP[p * 128 + 64:p * 128 + 96] = \
            woT[32 * (2 * p + 1):32 * (2 * p + 1) + 32]
    woP = _bf(woP)

    w1T = _bf(np.asarray(W1, np.float32).T)
    w2T = _bf(np.asarray(W2, np.float32).T)

    ind4 = np.zeros((8, 512), np.float32)
    for p in range(4):
        ind4[2 * p, p * 128 + 0:p * 128 + 32] = 1.0
        ind4[2 * p + 1, p * 128 + 64:p * 128 + 96] = 1.0
    ind4 = _bf(ind4)

    def gb(g, b):
        t = np.zeros((128, 4), np.float32)
        t[:, 0] = g[0:128]
        t[:, 1] = g[128:256]
        t[:, 2] = b[0:128]
        t[:, 3] = b[128:256]
        return t

    gb1 = gb(np.asarray(g1, np.float32), np.asarray(b1, np.float32))
    gb2 = gb(np.asarray(g2, np.float32), np.asarray(b2, np.float32))
    hT = _bf(h.T)

    in_maps = []
    for c in range(NCORES):
        sl = slice(c * NLOC, (c + 1) * NLOC)
        in_maps.append({
            "hT": np.ascontiguousarray(hT),
            "hq": np.ascontiguousarray(hT[:, sl]),
            "AT": np.ascontiguousarray(_bf(A[sl, :].T)),
            "wqT": wqT, "wkT": wkT, "wvT": wvT, "woP": woP,
            "w1T": w1T, "w2T": w2T, "ind4": ind4,
            "gb1": gb1, "gb2": gb2,
        })
    return in_maps


def build_nc(ncores=NCORES):
    nc = bacc.Bacc("TRN2", target_bir_lowering=False, debug=False,
                   num_devices=ncores)
    d = {}

    def inp(name, shape, dt):
        d[name] = nc.dram_tensor(name, shape, dt, kind="ExternalInput")

    inp("hT", [D, N], BF16)
    inp("hq", [D, NLOC], BF16)
    inp("AT", [N, NLOC], BF16)
    inp("wqT", [D, D], BF16)
    inp("wkT", [D, D], BF16)
    inp("wvT", [D, D], BF16)
    inp("woP", [512, D], BF16)
    inp("w1T", [D, 2 * D], BF16)
    inp("w2T", [2 * D, D], BF16)
    inp("ind4", [8, 512], BF16)
    inp("gb1", [128, 4], F32)
    inp("gb2", [128, 4], F32)
    yT = nc.dram_tensor("yT", [D, NLOC], F32, kind="ExternalOutput")

    with tile.TileContext(nc) as tc:
        _emit(nc, tc, d, yT, ncores)
    nc.compile()
    return nc


def _emit(nc, tc, d, yT, ncores):
    with (tc.tile_pool(name="const", bufs=1) as const,
          tc.tile_pool(name="work", bufs=3) as work,
          tc.tile_pool(name="fge", bufs=6) as fge,
          tc.tile_pool(name="atp", bufs=3) as atp,
          tc.tile_pool(name="dram", bufs=1, space="DRAM") as dram):

        # ---------- load h + weights (DMAs split across both HWDGE queues)
        hT_sb = [const.tile([P, N], BF16, name=f"hT{i}") for i in range(2)]
        for i in range(2):
            for j in range(4):
                eng = nc.sync if (i * 4 + j) % 2 == 0 else nc.scalar
                eng.dma_start(hT_sb[i][:, 1024 * j:1024 * (j + 1)],
                              d["hT"].ap()[P * i:P * (i + 1),
                                           1024 * j:1024 * (j + 1)])
        hq_sb = [const.tile([P, NLOC], BF16, name=f"hq{i}") for i in range(2)]
        for i in range(2):
            nc.sync.dma_start(hq_sb[i][:], d["hq"].ap()[P * i:P * (i + 1), :])

        def wtiles(name, ncol, nt):
            ts = []
            for i in range(nt):
                t = const.tile([P, ncol], BF16, name=f"{name}{i}")
                nc.scalar.dma_start(t[:], d[name].ap()[P * i:P * (i + 1), :])
                ts.append(t)
            return ts

        wk_sb = wtiles("wkT", D, 2)
        wv_sb = wtiles("wvT", D, 2)
        wq_sb = wtiles("wqT", D, 2)

        kT_sb = [const.tile([P, N], BF16, name=f"kT{g}") for g in range(2)]
        v_sb = [const.tile([P, 512], BF16, name=f"v{mc}") for mc in range(MCH)]
        qT_sb = [const.tile([P, NLOC], BF16, name=f"qT{g}") for g in range(2)]

        # ---------- projections ----------
        with tc.tile_pool(name="psW1", bufs=3, space="PSUM") as psW1:
            for g in range(2):
                for j in range(N // 512):
                    ps = psW1.tile([P, 512], F32, tag="ps512", name="ps512")
                    for kc in range(2):
                        nc.tensor.matmul(
                            ps[:], wk_sb[kc][:, P * g:P * (g + 1)],
                            hT_sb[kc][:, 512 * j:512 * (j + 1)],
                            start=(kc == 0), stop=(kc == 1))
                    nc.vector.tensor_copy(kT_sb[g][:, 512 * j:512 * (j + 1)],
                                          ps[:])
            for mc in range(MCH):
                ps = psW1.tile([P, 512], F32, tag="ps512", name="ps512")
                for kc in range(2):
                    nc.tensor.matmul(ps[0:P, 0:D],
                                     hT_sb[kc][:, P * mc:P * (mc + 1)],
                                     wv_sb[kc][:],
                                     start=(kc == 0), stop=(kc == 1))
                # cols 33-63 of each 64-block stay uninitialized: they only
                # produce U rows that are never read downstream.
                vv = v_sb[mc][:].rearrange("p (h c) -> p h c", c=64)
                nc.scalar.copy(vv[:, :, 0:32],
                               ps[0:P, 0:D].rearrange("p (h c) -> p h c",
                                                      c=32))
                nc.vector.memset(vv[:, :, 32:33], 1.0)
            for g in range(2):
                ps = psW1.tile([P, 512], F32, tag="ps512", name="ps512")
                for kc in range(2):
                    nc.tensor.matmul(ps[:], wq_sb[kc][:, P * g:P * (g + 1)],
                                     hq_sb[kc][:],
                                     start=(kc == 0), stop=(kc == 1))
                nc.scalar.copy(qT_sb[g][:], ps[:])

        # colsum(V) accumulated pre-loop while PSUM is free
        ones_t = const.tile([P, 1], BF16, name="ones")
        nc.vector.memset(ones_t[:], 1.0)
        cs32 = const.tile([1, 512], F32, name="cs32")
        with tc.tile_pool(name="psCS", bufs=1, space="PSUM") as psCS:
            cs_ps = psCS.tile([P, 512], F32, name="cs")
            for mc in range(MCH):
                nc.tensor.matmul(cs_ps[0:1, :], ones_t[:], v_sb[mc][:],
                                 start=(mc == 0), stop=(mc == MCH - 1))
            nc.vector.tensor_copy(cs32[:], cs_ps[0:1, :])

        vneg_sb = [const.tile([P, 512], BF16, name=f"vn{mc}")
                   for mc in range(MCH)]
        for mc in range(MCH):
            nc.gpsimd.tensor_scalar_mul(vneg_sb[mc][:], v_sb[mc][:], -1.0)

        # ---------- attention m-loop ----------
        with tc.tile_pool(name="psU", bufs=1, space="PSUM") as psU:
            u_ps = [psU.tile([P, NLOC], F32, tag=f"u{p}", name=f"u{p}")
                    for p in range(4)]

            with tc.tile_pool(name="psA", bufs=2, space="PSUM") as psA:
                for mc in range(MCH):
                    at_t = atp.tile([P, NLOC], BF16, tag="at", name="at")
                    eng = nc.sync if mc % 2 == 0 else nc.scalar
                    eng.dma_start(at_t[:],
                                  d["AT"].ap()[P * mc:P * (mc + 1), :])

                    # -V^T A term: one full-array matmul per pair; runs
                    # first so mc==0 carries start=True for the bank.
                    for p in range(4):
                        nc.tensor.matmul(
                            u_ps[p][:],
                            vneg_sb[mc][:, 128 * p:128 * (p + 1)],
                            at_t[:], start=(mc == 0), stop=False,
                            skip_group_check=True)
                    g_ts = []
                    for p in range(4):
                        grp = p // 2
                        i0 = (2 * p) % 4
                        sc = psA.tile([P, 1024], F32, tag="score",
                                      name="score")
                        for k in range(2):
                            i = i0 + k
                            nc.tensor.matmul(
                                sc[:, 512 * k:512 * (k + 1)],
                                kT_sb[grp][32 * i:32 * (i + 1),
                                           P * mc:P * (mc + 1)],
                                qT_sb[grp][32 * i:32 * (i + 1), :],
                                start=True, stop=True,
                                tile_position=(32 * i, 0))
                        ft = fge.tile([P, 1024], BF16, tag="f", name="f")
                        nc.scalar.activation(ft[:], sc[:], ACTF.Exp)
                        g_t = fge.tile([P, 1024], BF16, tag="g", name="g")
                        at_b = at_t[:].rearrange("p (o n) -> p o n", o=1
                                                 ).broadcast_to((P, 2, NLOC))
                        nc.vector.tensor_tensor(
                            g_t[:].rearrange("p (k n) -> p k n", k=2),
                            ft[:].rearrange("p (k n) -> p k n", k=2),
                            at_b, op=OP.mult)
                        g_ts.append(g_t)
                    for p in range(4):
                        for k, (base, tp) in enumerate(((0, (0, 0)),
                                                        (64, (0, 64)))):
                            hh = 2 * p + k
                            nc.tensor.matmul(
                                u_ps[p][base:base + 64, :],
                                v_sb[mc][:, 64 * hh:64 * hh + 64],
                                g_ts[p][:, 512 * k:512 * (k + 1)],
                                start=False, stop=(mc == MCH - 1),
                                tile_position=tp, skip_group_check=True)

            # ---------- Z, normalize, Wo, BN1, FFN, BN2 ----------
            with tc.tile_pool(name="psW2", bufs=2, space="PSUM") as psW2:
                csPt = const.tile([P, 4], F32, name="csPt")
                for pp in range(4):
                    nc.sync.dma_start(csPt[:, pp:pp + 1],
                                      cs32[0:1, P * pp:P * (pp + 1)])

                ucorr = [const.tile([P, NLOC], F32, name=f"uc{p}")
                         for p in range(4)]
                for p in range(4):
                    nc.vector.tensor_scalar(ucorr[p][:], u_ps[p][:],
                                            csPt[:, p:p + 1], None,
                                            op0=OP.add)

                z_sb = const.tile([8, NLOC], F32, name="z")
                for p in range(4):
                    nc.sync.dma_start(z_sb[2 * p:2 * p + 1, :],
                                      ucorr[p][32:33, :])
                    nc.sync.dma_start(z_sb[2 * p + 1:2 * p + 2, :],
                                      ucorr[p][96:97, :])
                r_sb = const.tile([8, NLOC], F32, name="r")
                nc.vector.reciprocal(r_sb[:], z_sb[:])
                r_bf = const.tile([8, NLOC], BF16, name="rb")
                nc.vector.tensor_copy(r_bf[:], r_sb[:])

                ind_sb = const.tile([8, 512], BF16, name="ind")
                nc.sync.dma_start(ind_sb[:], d["ind4"].ap())

                uT = []
                for p in range(4):
                    rexp = psW2.tile([P, NLOC], F32, tag="ps512",
                                     name="rexp")
                    nc.tensor.matmul(rexp[:], ind_sb[:, P * p:P * (p + 1)],
                                     r_bf[:], start=True, stop=True)
                    ut = const.tile([P, NLOC], BF16, name=f"ut{p}")
                    nc.vector.tensor_tensor(ut[:], ucorr[p][:], rexp[:],
                                            op=OP.mult)
                    uT.append(ut)

                wo_sb = [const.tile([P, D], BF16, name=f"wo{p}")
                         for p in range(4)]
                for p in range(4):
                    nc.sync.dma_start(wo_sb[p][:],
                                      d["woP"].ap()[P * p:P * (p + 1), :])
                gb1_sb = const.tile([P, 4], F32, name="gb1")
                nc.sync.dma_start(gb1_sb[:], d["gb1"].ap())
                gb2_sb = const.tile([P, 4], F32, name="gb2")
                nc.sync.dma_start(gb2_sb[:], d["gb2"].ap())
                eps_t = const.tile([P, 1], F32, name="eps")
                nc.vector.memset(eps_t[:], EPS)

                y1 = []
                for e in range(2):
                    psa = psW2.tile([P, NLOC], F32, tag="ps512", name="psa")
                    psb = psW2.tile([P, NLOC], F32, tag="ps512", name="psb")
                    for p in range(4):
                        nc.tensor.matmul(psa[:],
                                         wo_sb[p][0:32, P * e:P * (e + 1)],
                                         uT[p][0:32, :],
                                         start=(p == 0), stop=(p == 3),
                                         tile_position=(0, 0),
                                         skip_group_check=True)
                        nc.tensor.matmul(psb[:],
                                         wo_sb[p][64:96, P * e:P * (e + 1)],
                                         uT[p][64:96, :],
                                         start=(p == 0), stop=(p == 3),
                                         tile_position=(64, 0),
                                         skip_group_check=True)
                    yt = const.tile([P, NLOC], F32, name=f"y1_{e}")
                    nc.vector.tensor_tensor(yt[:], psa[:], hq_sb[e][:],
                                            op=OP.add)
                    nc.vector.tensor_tensor(yt[:], yt[:], psb[:], op=OP.add)
                    y1.append(yt)

                def batchnorm(y_tiles, gb_sb, out_dt, tag):
                    stats = const.tile([P, 4], F32, name=f"st_{tag}")
                    for e in range(2):
                        nc.vector.tensor_reduce(stats[:, 2 * e:2 * e + 1],
                                                y_tiles[e][:], AX.X, OP.add)
                        sq = work.tile([P, NLOC], BF16, tag="sq", name="sq")
                        nc.scalar.activation(
                            sq[:], y_tiles[e][:], ACTF.Square,
                            accum_out=stats[:, 2 * e + 1:2 * e + 2])
                    if ncores > 1:
                        b_in = dram.tile([P, 4], F32, name=f"bi_{tag}")
                        b_out = dram.tile([P * ncores, 4], F32,
                                          addr_space="Shared",
                                          name=f"bo_{tag}")
                        nc.sync.dma_start(b_in[:], stats[:])
                        nc.gpsimd.collective_compute(
                            "AllGather", OP.bypass, ins=[b_in[:].opt()],
                            outs=[b_out[:].opt()],
                            replica_groups=[list(range(ncores))])
                        gath = const.tile([P, 4 * ncores], F32,
                                          name=f"ga_{tag}")
                        nc.sync.dma_start(
                            gath[:],
                            b_out[:].rearrange("(g p) s -> p g s", p=P))
                        tot = const.tile([P, 4], F32, name=f"tot_{tag}")
                        gv = gath[:].rearrange("p (g s) -> p s g", s=4)
                        for s in range(4):
                            nc.vector.tensor_reduce(tot[:, s:s + 1],
                                                    gv[:, s, :], AX.X,
                                                    OP.add)
                    else:
                        tot = stats
                    outs = []
                    for e in range(2):
                        mean = const.tile([P, 1], F32, name=f"mn{e}_{tag}")
                        nc.vector.tensor_scalar(mean[:],
                                                tot[:, 2 * e:2 * e + 1],
                                                1.0 / N, None, op0=OP.mult)
                        m2 = const.tile([P, 1], F32, name=f"m2{e}_{tag}")
                        nc.vector.tensor_tensor(m2[:], mean[:], mean[:],
                                                op=OP.mult)
                        var = const.tile([P, 1], F32, name=f"vr{e}_{tag}")
                        nc.vector.scalar_tensor_tensor(
                            var[:], tot[:, 2 * e + 1:2 * e + 2], 1.0 / N,
                            m2[:], op0=OP.mult, op1=OP.subtract)
                        std = const.tile([P, 1], F32, name=f"sd{e}_{tag}")
                        nc.scalar.activation(std[:], var[:], ACTF.Sqrt,
                                             bias=eps_t[:])
                        rstd = const.tile([P, 1], F32, name=f"rs{e}_{tag}")
                        nc.vector.reciprocal(rstd[:], std[:])
                        alpha = const.tile([P, 1], F32, name=f"al{e}_{tag}")
                        nc.vector.tensor_tensor(alpha[:], gb_sb[:, e:e + 1],
                                                rstd[:], op=OP.mult)
                        nma = const.tile([P, 1], F32, name=f"nm{e}_{tag}")
                        nc.vector.scalar_tensor_tensor(
                            nma[:], mean[:], -1.0, alpha[:],
                            op0=OP.mult, op1=OP.mult)
                        beta = const.tile([P, 1], F32, name=f"be{e}_{tag}")
                        nc.vector.tensor_tensor(beta[:],
                                                gb_sb[:, e + 2:e + 3],
                                                nma[:], op=OP.add)
                        o = const.tile([P, NLOC], out_dt,
                                       name=f"bn{e}_{tag}")
                        nc.vector.tensor_scalar(o[:], y_tiles[e][:],
                                                alpha[:], beta[:],
                                                op0=OP.mult, op1=OP.add)
                        outs.append(o)
                    return outs

                bn1 = batchnorm(y1, gb1_sb, BF16, "bn1")

                w1_sb = [const.tile([P, 2 * D], BF16, name=f"w1_{i}")
                         for i in range(2)]
                for i in range(2):
                    nc.sync.dma_start(w1_sb[i][:],
                                      d["w1T"].ap()[P * i:P * (i + 1), :])
                w2_sb = [const.tile([P, D], BF16, name=f"w2_{i}")
                         for i in range(4)]
                for i in range(4):
                    nc.sync.dma_start(w2_sb[i][:],
                                      d["w2T"].ap()[P * i:P * (i + 1), :])

                z_t = []
                for m in range(4):
                    ps = psW2.tile([P, NLOC], F32, tag="ps512", name="ffn1")
                    for kc in range(2):
                        nc.tensor.matmul(ps[:],
                                         w1_sb[kc][:, P * m:P * (m + 1)],
                                         bn1[kc][:],
                                         start=(kc == 0), stop=(kc == 1))
                    zt = const.tile([P, NLOC], BF16, name=f"z{m}")
                    nc.vector.tensor_scalar_max(zt[:], ps[:], 0.0)
                    z_t.append(zt)

                y2 = []
                for e in range(2):
                    ps = psW2.tile([P, NLOC], F32, tag="ps512", name="ffn2")
                    for kc in range(4):
                        nc.tensor.matmul(ps[:],
                                         w2_sb[kc][:, P * e:P * (e + 1)],
                                         z_t[kc][:],
                                         start=(kc == 0), stop=(kc == 3))
                    yt = const.tile([P, NLOC], F32, name=f"y2_{e}")
                    nc.vector.tensor_tensor(yt[:], ps[:], bn1[e][:],
                                            op=OP.add)
                    y2.append(yt)

                bn2 = batchnorm(y2, gb2_sb, F32, "bn2")
                for e in range(2):
                    nc.sync.dma_start(yT.ap()[P * e:P * (e + 1), :],
                                      bn2[e][:])


_CACHE = {}


def kernel(A, h, Wq, Wk, Wv, Wo, g1, b1, g2, b2, W1, W2):
    from concourse import bass_utils
    if "nc" not in _CACHE:
        _CACHE["nc"] = build_nc(NCORES)
    in_maps = _prep_inputs(A, h, Wq, Wk, Wv, Wo, g1, b1, g2, b2, W1, W2)
    res = bass_utils.run_bass_kernel_spmd(_CACHE["nc"], in_maps,
                                          core_ids=list(range(NCORES)))
    return np.concatenate([np.asarray(r["yT"], np.float32).T
                           for r in res.results], axis=0)
